# revision 14
# baseline (speedup 1.0000x reference)
"""CVKAN layer kernel for Trainium2 (8 NeuronCores, data-parallel over batch).

Math (see reference):
    basis[b,i,u,v] = exp(-((x_re[b,i]-lin_u)^2 + (x_im[b,i]-lin_v)^2))
                   = eu[b,i,u] * ev[b,i,v]            (separable!)
    out_re[b,o]    = sum_{i,u,v} basis * RW[i,o,u,v] + silu terms
    out_im[b,o]    = sum_{i,u,v} basis * CW[i,o,u,v] + silu terms

Per-core layout is transposed (contraction dim on partitions):
    euT[t][ki, u, b], evT[t][ki, v, b]   with i = t*128 + ki, b = 512 rows/core
    P_{t,u,v}[ki, b] = euT[t][ki,u,b] * evT[t][ki,v,b]       (DVE, bf16)
    psum[mo][o, b] += W[u,t,v,mo][ki, o].T @ P_{t,u,v}       (PE, bf16)
over all (u, t, v): 64 (u,v) pairs x 2 k-tiles x 4 output tiles = 512 matmuls,
plus 16 silu matmuls, accumulated in 4 PSUM banks (512 out channels = re|im).

The PE path runs in bf16 (weights, basis products, silu factors): halves the
weight stream vs fp32, enables fast-weight-load, doubles DVE throughput for
the basis products; error ~3e-3 vs the 2e-2 gate.

For the timing loop the body is UNROLLED x2 with ping-pong buffers (PSUM
bank sets 0-3/4-7; alternate x/eu/ev/sre/sim SBUF sets).  Rationale: with a
single body, each engine's stream contains at least one instruction that
WAR-blocks on the very END of the previous iteration (PSUM drains, silu
factor writes, sw reloads, out DMAs), which stops every engine from racing
ahead across the loop edge and starves the PE for ~25us/iter.  With the
ping-pong, WARs point at the iteration-before-last, so ACT/DVE/Pool/SP all
run a full iteration ahead of the PE.
"""

import contextlib
import numpy as np
import ml_dtypes

import concourse.bass as bass
import concourse.bacc as bacc
import concourse.mybir as mybir
from concourse import tile
from concourse.bass_utils import run_bass_kernel_spmd

B, I, O, G = 4096, 256, 256, 8
GRID_MIN, GRID_MAX, RHO = -2.0, 2.0, 1.0
NCORES = 8
BS = B // NCORES          # 512 batch rows per core
OC = 2 * O                # 512 combined output channels (re | im)
KT = I // 128             # 2 k-tiles
MO = OC // 128            # 4 output partition tiles
N_WARM = 10               # PE warmup matmuls (outside the timing loop)

F32 = mybir.dt.float32
BF16 = mybir.dt.bfloat16
AF = mybir.ActivationFunctionType

LIN = np.linspace(GRID_MIN, GRID_MAX, G, dtype=np.float32)
BF = ml_dtypes.bfloat16


def build_nc(loops=None):
    """loops: if set, wrap the per-call body in a device-side For_i — used only
    by the timing harness to amortize host/axon dispatch overhead.  `loops`
    counts logical iterations; the hardware loop runs loops//2 double-bodies."""
    unroll = 2 if loops else 1
    nc = bacc.Bacc("TRN2", target_bir_lowering=False, debug=False,
                   num_devices=NCORES)

    xt_re = nc.declare_dram_parameter("xt_re", [I, BS], F32, isOutput=False)
    xt_im = nc.declare_dram_parameter("xt_im", [I, BS], F32, isOutput=False)
    # (t, u, ki, v, mo, o): one 8KB/partition DMA per (t, u)
    w = nc.declare_dram_parameter("w", [KT, G, 128, G, MO, 128], BF16,
                                  isOutput=False)
    # (m, t, ki, mo, o)
    sw = nc.declare_dram_parameter("sw", [2, KT, 128, MO, 128], BF16,
                                   isOutput=False)
    bias = nc.declare_dram_parameter("bias", [MO, 128, 1], F32, isOutput=False)
    out = nc.declare_dram_parameter("out", [OC, BS], F32, isOutput=True)

    with tile.TileContext(nc) as tc:
        with (
            tc.tile_pool(name="cpool", bufs=1) as cpool,
            tc.tile_pool(name="wpool", bufs=4) as wpool,
            tc.tile_pool(name="ppool", bufs=4) as ppool,
            tc.tile_pool(name="sqpool", bufs=12) as sqpool,
            tc.tile_pool(name="sgpool", bufs=4) as sgpool,
            tc.tile_pool(name="pspool", bufs=1, space="PSUM") as pspool,
            tc.tile_pool(name="opool", bufs=4) as opool,
        ):
            # ---- persistent SBUF tensors, one set per unroll phase ----
            def per_phase(shape, dt, nm):
                return [[cpool.tile(shape, dt, name=f"{nm}{t}p{ph}",
                                    tag=f"{nm}{t}p{ph}") for t in range(KT)]
                        for ph in range(unroll)]

            xtr = per_phase([128, BS], F32, "xtr")
            xti = per_phase([128, BS], F32, "xti")
            eu = per_phase([128, G, BS], BF16, "eu")
            ev = per_phase([128, G, BS], BF16, "ev")
            sre = per_phase([128, BS], BF16, "sre")
            sim_ = per_phase([128, BS], BF16, "sim")
            swt = [[cpool.tile([128, MO, 128], BF16, name=f"sw{m}{t}",
                               tag=f"sw{m}{t}")
                    for t in range(KT)] for m in range(2)]
            bt = [cpool.tile([128, 1], F32, name=f"bias{mo}", tag=f"bias{mo}")
                  for mo in range(MO)]
            psum = [[pspool.tile([128, BS], F32, name=f"acc{mo}p{ph}",
                                 tag=f"acc{mo}p{ph}") for mo in range(MO)]
                    for ph in range(unroll)]
            negl = [cpool.tile([128, 1], F32, name=f"negl{g}", tag=f"negl{g}")
                    for g in range(G)]

            # ---- one-time constants + PE warmup (outside the timing loop) ----
            junk = cpool.tile([128, BS], BF16, name="junk", tag="junk")
            nc.gpsimd.memset(junk[:], 1.0)
            for g in range(G):
                nc.gpsimd.memset(negl[g][:], -float(LIN[g]))
            for m in range(2):
                for t in range(KT):
                    nc.gpsimd.dma_start(out=swt[m][t][:], in_=sw[m, t])
            for mo in range(MO):
                nc.gpsimd.dma_start(out=bt[mo][:], in_=bias[mo])
            for _ in range(N_WARM):
                nc.tensor.matmul(psum[0][0][:], junk[:, 0:128], junk[:],
                                 start=True, stop=True, skip_group_check=True)

            def emit_iter(ph):
                xr, xi = xtr[ph], xti[ph]
                eup, evp = eu[ph], ev[ph]
                srp, sip = sre[ph], sim_[ph]
                psp = psum[ph]

                # ---- input DMAs.  x rides SP in front of this phase's
                # weight stream; sw/bias reload on Pool once per body. ----
                for t in range(KT):
                    nc.sync.dma_start(out=xr[t][:],
                                      in_=xt_re[t * 128:(t + 1) * 128, :])
                    nc.sync.dma_start(out=xi[t][:],
                                      in_=xt_im[t * 128:(t + 1) * 128, :])
                wt0 = wpool.tile([128, G, MO, 128], BF16, name="wt0",
                                 tag="wt")
                nc.sync.dma_start(out=wt0[:], in_=w[0, 0])

                # ---- RBF factors:  e = exp(-(x - lin_g)^2 / RHO), all on
                # ACT (Square then Exp); ACT has no end-of-iteration WARs so
                # it races ahead freely. ----
                def rbf(dst, src, g):
                    sq = sqpool.tile([128, BS], F32, name="sq", tag="sq")
                    nc.scalar.activation(sq[:], src[:], AF.Square,
                                         bias=negl[g][:])
                    nc.scalar.activation(dst, sq[:], AF.Exp, scale=-1.0 / RHO)

                rbf(evp[0][:, 0, :], xi[0], 0)
                rbf(eup[0][:, 0, :], xr[0], 0)
                for v in range(1, G):
                    rbf(evp[0][:, v, :], xi[0], v)
                for u in range(1, G):
                    rbf(eup[0][:, u, :], xr[0], u)
                rbf(evp[1][:, 0, :], xi[1], 0)
                rbf(eup[1][:, 0, :], xr[1], 0)
                for v in range(1, G):
                    rbf(evp[1][:, v, :], xi[1], v)
                for u in range(1, G):
                    rbf(eup[1][:, u, :], xr[1], u)

                # ---- silu factors: single fused ACT op; the sre/sim
                # ping-pong keeps the WAR off the loop edge. ----
                def silu(src, dst):
                    nc.scalar.activation(dst[:], src[:], AF.Silu)

                silu(xr[0], srp[0])
                silu(xi[0], sip[0])
                silu(xr[1], srp[1])
                silu(xi[1], sip[1])

                # ---- main contraction ----
                for t in range(KT):
                    for u in range(G):
                        p = ppool.tile([128, G, BS], BF16, name="p", tag="p")
                        if t == 0 and u <= 1:
                            # per-v products: the early matmuls only need the
                            # ev slices ACT has produced so far (single-shot
                            # startup; in the loop everything is prebuilt)
                            for v in range(G):
                                nc.vector.tensor_mul(p[:, v, :],
                                                     eup[t][:, u, :],
                                                     evp[t][:, v, :])
                        else:
                            nc.vector.tensor_mul(
                                p[:],
                                eup[t][:, u:u + 1, :].to_broadcast((128, G, BS)),
                                evp[t][:],
                            )
                        if t == 0 and u == 0:
                            wt = wt0
                        else:
                            wt = wpool.tile([128, G, MO, 128], BF16,
                                            name="wt", tag="wt")
                            nc.sync.dma_start(out=wt[:], in_=w[t, u])
                        for v in range(G):
                            for mo in range(MO):
                                nc.tensor.matmul(
                                    psp[mo][:],
                                    wt[:, v, mo, :],
                                    p[:, v, :],
                                    start=(u == 0 and t == 0 and v == 0),
                                    stop=False,
                                )

                # ---- silu matmuls, mo-outer so psum banks finish staggered.
                # DVE drains each finished bank (Pool cannot access PSUM);
                # out-DMA issues ride the Pool queue. ----
                for mo in range(MO):
                    for m in range(2):
                        s = srp if m == 0 else sip
                        for t in range(KT):
                            nc.tensor.matmul(
                                psp[mo][:],
                                swt[m][t][:, mo, :],
                                s[t][:],
                                start=False,
                                stop=(m == 1 and t == KT - 1),
                            )
                    ot = opool.tile([128, BS], F32, name=f"ot{mo}", tag="ot")
                    nc.vector.tensor_scalar_add(ot[:], psp[mo][:], bt[mo][:])
                    nc.gpsimd.dma_start(out=out[mo * 128:(mo + 1) * 128, :],
                                        in_=ot[:])

            if loops:
                with tc.For_i(0, loops // unroll, 1):
                    for ph in range(unroll):
                        emit_iter(ph)
            else:
                emit_iter(0)

    nc.finalize()
    return nc


def prep_inputs(x_re, x_im, realweights, complexweights,
                silu_weight_re, silu_weight_im, silu_bias_re, silu_bias_im):
    """Host-side shard/layout prep. Returns in_maps for the 8 cores."""
    x_re = np.ascontiguousarray(x_re, np.float32)
    x_im = np.ascontiguousarray(x_im, np.float32)

    # (I, O', u, v) -> (t, u, ki, v, mo, o)
    wc = np.concatenate([np.asarray(realweights, np.float32),
                         np.asarray(complexweights, np.float32)], axis=1)
    w_dev = np.ascontiguousarray(
        wc.reshape(KT, 128, MO, 128, G, G)
        .transpose(0, 4, 1, 5, 2, 3).astype(BF))

    swr = np.asarray(silu_weight_re, np.float32)
    swi = np.asarray(silu_weight_im, np.float32)
    # out_re += s_re@swr - s_im@swi ; out_im += s_re@swi + s_im@swr
    sw1 = np.concatenate([swr, swi], axis=1)      # multiplies s_re
    sw2 = np.concatenate([-swi, swr], axis=1)     # multiplies s_im
    sw_dev = np.ascontiguousarray(
        np.stack([sw1, sw2]).reshape(2, KT, 128, MO, 128).astype(BF))

    bias_dev = np.ascontiguousarray(
        np.concatenate([np.asarray(silu_bias_re, np.float32).sum(0),
                        np.asarray(silu_bias_im, np.float32).sum(0)])
        .reshape(MO, 128, 1))

    in_maps = []
    for c in range(NCORES):
        sl = slice(c * BS, (c + 1) * BS)
        in_maps.append({
            "xt_re": np.ascontiguousarray(x_re[sl].T),
            "xt_im": np.ascontiguousarray(x_im[sl].T),
            "w": w_dev,
            "sw": sw_dev,
            "bias": bias_dev,
        })
    return in_maps


def assemble_output(results):
    out = np.empty((B, O, 2), np.float32)
    for c in range(NCORES):
        t = results[c]["out"]               # (OC, BS)
        sl = slice(c * BS, (c + 1) * BS)
        out[sl, :, 0] = t[:O].T
        out[sl, :, 1] = t[O:].T
    return out


_NC = None


def run(inputs, **spmd_kwargs):
    """Run on the 8 cores; returns (full_output, BassKernelResults)."""
    global _NC
    if _NC is None:
        _NC = build_nc()
    in_maps = prep_inputs(**inputs)
    res = run_bass_kernel_spmd(_NC, in_maps, list(range(NCORES)), **spmd_kwargs)
    return assemble_output(res.results), res


def kernel(**inputs) -> np.ndarray:
    out, _ = run(inputs)
    return out


if __name__ == "__main__":
    import reference
    inputs = {k: np.asarray(v) for k, v in reference.setup_inputs().items()}
    expected = np.asarray(reference.reference(**inputs))
    actual = kernel(**inputs)
    err = np.abs(actual - expected).max() / np.abs(expected).max()
    print("Relative error:", err)
